# revision 50
# baseline (speedup 1.0000x reference)
"""DeepHisCoM Trainium2 kernel (nn_DeepHisCoM_7017976562218).

Math (reference):
    xr = x.reshape(B, P, V)
    z1 = einsum('bpv,pwv->bpw', xr, W1);  h = leaky(z1)          # per-pathway Linear V->W
    z2 = einsum('bpw,pw->bp', h, W2);     pval = leaky(z2)       # per-pathway Linear W->1
    BN(batch stats) -> global L2 normalize -> sigmoid(pn @ Wd + bd)

Device strategy (8 NeuronCores, batch-sharded 2048 rows/core):
    - x is pre-transposed and fp8(e4m3)-quantized on the HOST into
      [bt, v, pathway, batch] layout, so TensorE consumes it directly as
      matmul lhsT (no on-chip transposes, no transpose PSUM->SBUF copies,
      and half the HBM bytes vs bf16).
    - One 66-column matmul per pathway: rhs = [4*W1p^T | +16u | -16u] (fp8)
      with u = 0.2 * W1p^T @ W2p.  leaky(z1) = 0.2*z1 + 0.8*relu(z1), so
      z2 = sum_w relu(z1)*0.8*W2 + (relu(q) - relu(-q)) with q = 0.2*sum_w
      z1*W2 carried exactly by the +/-u columns (relu is positively
      homogeneous, so the 4x/16x fp8-range scales cancel against w2e).
    - Matmuls write f32 to PSUM, 7 pathways per bank, 14 per 2-bank tile.
    - VectorE: fused prefix-scan DVE op (running sum of w2e * relu(h),
      continuous across the group); per-pathway sums are recovered as
      differences of the segment-end columns.  The op carries a
      hand-authored 2X_1PORT microcode variant (2 bf16 pairs/cycle); for
      6 of 10 groups per tile ScalarE stages PSUM f32 -> SBUF bf16 so the
      scan runs at 2x, and those groups are scanned merged 3-at-a-time.
      The remaining groups scan straight from PSUM f32 at 1x, balancing
      Vector vs Scalar load.
    - Ends extraction (ScalarE strided copy), boundary diffs (GpSimd),
      and the final leaky (VectorE) are software-pipelined one batch tile
      behind the feeder ops so no engine queue head-of-line blocks the
      next tile.
    - BN stats + L2 norm + final linear + sigmoid on host (8 MiB, trivial).

fp8 is safe here: BN renormalizes each pathway and the global L2 norm +
sigmoid-around-0.5 crush relative noise; measured rel err stays ~8e-6.
"""

import os
import sys

import numpy as np

for _p in ("/opt/trn_rl_repo",):
    if _p not in sys.path and os.path.isdir(_p):
        sys.path.insert(0, _p)

import ml_dtypes

import concourse.bacc as bacc
import concourse.bass as bass
import concourse.mybir as mybir
from concourse import dve_ops
from concourse.bass_utils import run_bass_kernel_spmd
from concourse.dve_spec import AluOp, Spec, Src0, Src1, Zero, lower, relu, scan
from concourse.dve_uop import (
    ENABLE,
    AluInp,
    DelayInp,
    DveOpSpec,
    InpSel,
    OutPath,
    OutSel,
    Trigger,
    UopConfig,
)
from concourse.dve_uop import AluOp as HwAluOp
from concourse.tile import TileContext


def _build_2x_program():
    """2X_1PORT uop program for the prefix scan: each cycle consumes a
    packed bf16 pair from each source port (w_e,w_o / h_e,h_o), updates the
    accumulator by w_e*relu(h_e) + w_o*relu(h_o), and writes the post-pair
    prefix to BOTH 16-bit output halves.  Even output positions therefore
    hold the pair prefix instead of the element prefix — harmless, since
    only the odd segment-end columns are ever read.

    Mirrors the lower()-generated 1x program's FSM: state0 = one COUNT
    cycle that zeroes the accumulator flop, state1 = steady until
    SRC_TENSOR_DONE.
    """
    lanes = (
        (1, InpSel.SRC_0),      # chain 0: w_e
        (2, InpSel.SRC_1),      # chain 1: h_e
        (3, InpSel.ZERO),       # chain 2: 0.0 (relu operand / acc init)
        (4, InpSel.SRC_0_HI),   # chain 3: w_o
        (5, InpSel.SRC_1_HI),   # chain 4: h_o
    )

    # state 0: one counted cycle, no source consumption, zero the acc flop
    u0 = UopConfig()
    for lane, src in lanes:
        u0.enable_input(src, lane)
    for i in range(5):
        u0.datapath_config[i].pass_through_alu().pass_through_delay(2)
    u0.datapath_config[5].enable_alu(HwAluOp.BYPASS, AluInp.PREV_DELAY_2)
    u0.datapath_config[6].pass_through_alu()
    u0.datapath_config[7].pass_through_alu()
    u0.repeat_count = 1
    u0.trigger = (Trigger.COUNT, Trigger.NONE, Trigger.NONE)
    u0.next_uop = (1, 0, 0)

    # state 1: steady pair loop
    u1 = UopConfig()
    for lane, src in lanes:
        u1.enable_input(src, lane)
    u1.require_inp0 = ENABLE
    u1.require_inp1 = ENABLE
    u1.trigger = (Trigger.SRC_TENSOR_DONE, Trigger.NONE, Trigger.NONE)
    u1.next_uop = (0, 0, 0)
    u1.enable_output(OutSel.ALU_OUT, OutPath.WR0_LO)
    u1.enable_output(OutSel.ALU_OUT, OutPath.WR0_HI)
    b = u1.datapath_config
    # b0: relu_e = max(h_e, 0)
    b[0].enable_alu(HwAluOp.MAX, AluInp.PREV_DELAY_1, AluInp.PREV_DELAY_2)
    b[0].pass_through_delay(0, 2, 3, 4)
    # b1: m_e = w_e * relu_e
    b[1].enable_alu(HwAluOp.MULTIPLY, AluInp.PREV_DELAY_0, AluInp.PREV_ALU_OUT)
    b[1].pass_through_delay(2, 3, 4)
    # b2: relu_o = max(h_o, 0); chain0 <- m_e
    b[2].enable_alu(HwAluOp.MAX, AluInp.PREV_DELAY_4, AluInp.PREV_DELAY_2)
    b[2].enable_delay_from_src(DelayInp.PREV_ALU_OUT, 0)
    b[2].pass_through_delay(3)
    # b3: m_o = w_o * relu_o
    b[3].enable_alu(HwAluOp.MULTIPLY, AluInp.PREV_DELAY_3, AluInp.PREV_ALU_OUT)
    b[3].pass_through_delay(0)
    # b4: pair = m_o + m_e
    b[4].enable_alu(HwAluOp.ADD, AluInp.PREV_ALU_OUT, AluInp.PREV_DELAY_0)
    # b5: acc += pair  (CURR_ALU_OUT feedback, zeroed by state 0)
    b[5].enable_alu(HwAluOp.ADD, AluInp.CURR_ALU_OUT, AluInp.PREV_ALU_OUT)
    # b6/b7: forward acc to the write stage
    b[6].pass_through_alu()
    b[7].pass_through_alu()
    return [u0, u1]


class _DveOp2x(dve_ops.DveOp):
    """DveOp whose compiled spec carries a hand-authored 2X_1PORT program."""

    def compile(self, ver):
        key = (self.name, ver)
        if (r := dve_ops._COMPILE_CACHE.get(key)) is not None:
            return r
        result = DveOpSpec(
            name=self.name,
            opcode=dve_ops.get_dve_sub_opcode(self.name),
            uops=lower(self.spec, ver=ver),
            rd1_en=True,
            uops_2x=_build_2x_program() if ver == "v3" else None,
            perf_max=1,
        )
        got = result.sha(ver)
        if self.uops_sha.get(ver) != got:
            raise ValueError(
                f"{self.name}: lower() output drifted ({ver}: {got} != pinned "
                f"{self.uops_sha.get(ver)!r})."
            )
        dve_ops._COMPILE_CACHE[key] = result
        return result


def _register_prefix_sum_op():
    """Fused DVE op: out[t] = running sum of in0[t] * relu(in1[t]).

    Per-pathway sums are recovered afterwards as differences of the
    segment-boundary columns of the prefix sum.  Ships a hand-authored
    2X_1PORT uop variant (2 bf16 pairs/cycle) alongside the stock 1x one.
    """
    name = "STT_PREFIX_SUM_2X_ANT"
    for op in dve_ops.OPS:
        if op.name == name:
            return op

    def ref(in0, in1, s0, s1, imm2):
        return np.cumsum(in0.astype(np.float32) * np.maximum(in1, 0), axis=-1)

    spec = Spec(body=scan(AluOp.ADD, Src0 * relu(Src1), init=Zero), reference=ref)
    row = dve_ops._CUSTOM_DVE_ROW_BASE + len(dve_ops.OPS)
    sha = DveOpSpec(
        name=name,
        opcode=row,
        uops=lower(spec, ver="v3"),
        rd1_en=True,
        uops_2x=_build_2x_program(),
        perf_max=1,
    ).sha("v3")
    op = _DveOp2x(name, spec, subdim=False, uops_sha={"v3": sha})
    dve_ops.OPS.append(op)
    dve_ops._SUB_OPCODE_FOR_NAME[name] = row
    dve_ops.CUSTOM_DVE_SPECS[name] = op.spec
    return op


PREFIX_SUM_OP = _register_prefix_sum_op()

P, V, W = 128, 128, 64
B = 16384
N_CORES = 8
BSH = B // N_CORES          # 2048 batch rows per core
NBT = BSH // 128            # 16 batch tiles per core
BN_EPS = 1e-5
NCOL = W + 2                # 66: W1^T columns + (+u, -u)
F32 = mybir.dt.float32
BF16 = mybir.dt.bfloat16
FP8 = mybir.dt.float8e4
W1_SCALE = 4.0              # lift W1 out of fp8-subnormal range
U_SCALE = 16.0              # lift u columns out of fp8-subnormal range

# pathway groups per batch tile: 14 pathways per 2-bank PSUM tile (7 per
# 2 KB bank), 9 groups + a 2-pathway tail
GROUPS = [(g * 14, 14) for g in range(9)] + [(126, 2)]
NG = len(GROUPS)
ESTRIDE = 15                # ends-staging slots per group (leading zero + 14)
N_STAGED = 6                # groups per bt staged to SBUF bf16 (2x scan);
                            # the rest scan straight from PSUM f32 at 1x
NXCHUNK = 4                 # x-tile DMA split (compute starts on chunk 1)

_CACHE = {}
LAST_RESULTS = None


def _build_program():
    nc = bacc.Bacc()
    # row = bt*128 + v, col = pathway*128 + batch
    xt_in = nc.declare_dram_parameter("xt", [NBT * 128, P * 128], FP8, isOutput=False)
    wext_in = nc.declare_dram_parameter("wext", [V, P * NCOL], FP8, isOutput=False)
    w2e_in = nc.declare_dram_parameter("w2ext", [128, P * NCOL], BF16, isOutput=False)
    p_out = nc.declare_dram_parameter("ps", [BSH, P], F32, isOutput=True)

    with TileContext(nc) as tc:
        with (
            tc.tile_pool(name="singles", bufs=1) as singles,
            tc.tile_pool(name="xh", bufs=4) as xhp,
            tc.tile_pool(name="hsb", bufs=6) as hsbp,
            tc.tile_pool(name="sout", bufs=4) as soutp,
            tc.tile_pool(name="ends", bufs=3) as endsp,
            tc.tile_pool(name="pf", bufs=3) as pfp,
            tc.tile_pool(name="hps", bufs=4, space="PSUM") as hpsp,
        ):
            wext = singles.tile([V, P * NCOL], FP8)
            nc.sync.dma_start(out=wext[:], in_=wext_in[:, :])
            w2e = singles.tile([128, P * NCOL], BF16)
            # two chunks: the first merged scan only needs the low half
            nc.scalar.dma_start(
                out=w2e[:, : 64 * NCOL], in_=w2e_in[:, : 64 * NCOL]
            )
            nc.scalar.dma_start(
                out=w2e[:, 64 * NCOL :], in_=w2e_in[:, 64 * NCOL :]
            )

            CHW = P * 128 // NXCHUNK

            def load_x(eng, tile, bt):
                # chunked so the first matmul group can start after 1/NXCHUNK
                # of the tile has landed
                for ch in range(NXCHUNK):
                    eng.dma_start(
                        out=tile[:, ch * CHW : (ch + 1) * CHW],
                        in_=xt_in[bt * 128 : (bt + 1) * 128,
                                  ch * CHW : (ch + 1) * CHW],
                    )

            def emit_front(bt, xh):
                """matmuls + staging copies + scans for one batch tile."""
                # ends staging: slot g*15 stays 0 (leading zero per group)
                endsC = endsp.tile([128, NG * ESTRIDE], F32)
                nc.gpsimd.memset(endsC[:], 0.0)
                # per-bt scan output: group gi at column offset gi*924 (the
                # tail only fills 132 of its 924 slots; the rest is junk)
                sout = soutp.tile([128, NG * 14 * NCOL], BF16)
                hsb = None
                for gi, (gs, G) in enumerate(GROUPS):
                    g2 = (G + 1) // 2
                    h_ps = hpsp.tile([128, 1024], F32)
                    for j in range(G):
                        pa = gs + j
                        off = (j // g2) * 512 + (j % g2) * NCOL
                        nc.tensor.matmul(
                            h_ps[:, off : off + NCOL],
                            lhsT=xh[:, pa * 128 : (pa + 1) * 128],
                            rhs=wext[:, pa * NCOL : (pa + 1) * NCOL],
                            start=True,
                            stop=True,
                        )
                    if gi < N_STAGED:
                        # ScalarE: whole-tile PSUM f32 -> SBUF bf16 copy
                        # (contiguous, includes bank pads); the scan then
                        # runs in 2X_1PORT mode on packed bf16 pairs.
                        # Staged groups are scanned merged, 3 at a time.
                        if gi % 3 == 0:
                            hsb = hsbp.tile([128, 3 * 1024], BF16)
                        # only cols [0:974] are ever read by the scan (the
                        # last 50 are bank-1 tail padding) — skip them
                        nc.scalar.copy(
                            out=hsb[:, (gi % 3) * 1024 : (gi % 3) * 1024 + 974],
                            in_=h_ps[:, :974],
                        )
                        if gi % 3 == 2:
                            g0 = gi - 2
                            dve_inst = nc.vector._custom_dve(
                                PREFIX_SUM_OP,
                                out=sout[
                                    :, g0 * 14 * NCOL : (g0 + 3) * 14 * NCOL
                                ].rearrange("p (b c) -> p b c", b=6),
                                in0=w2e[
                                    :, g0 * 14 * NCOL : (g0 + 3) * 14 * NCOL
                                ].rearrange("p (b c) -> p b c", b=6),
                                in1=hsb[:].rearrange("p (b c) -> p b c", b=6)[
                                    :, :, : g2 * NCOL
                                ],
                            )
                            dve_inst.ins.perf_max = 1
                    else:
                        # unstaged: scan reads PSUM f32 directly; mode
                        # detection falls back to the 1x program
                        so = sout[:, gi * 14 * NCOL : gi * 14 * NCOL + G * NCOL]
                        dve_inst = nc.vector._custom_dve(
                            PREFIX_SUM_OP,
                            out=so.rearrange("p (b c) -> p b c", b=2),
                            in0=w2e[:, gs * NCOL : (gs + G) * NCOL].rearrange(
                                "p (b c) -> p b c", b=2
                            ),
                            in1=h_ps[:].rearrange("p (b c) -> p b c", b=2)[
                                :, :, : g2 * NCOL
                            ],
                        )
                        dve_inst.ins.perf_max = 1
                return sout, endsC

            def emit_tail(bt, sout, endsC):
                """ends extraction + diffs + leaky + store, emitted one bt
                late so these queue entries never block the next tile's
                feeder ops on the same engines."""
                # one strided ScalarE copy stages ALL segment-end columns:
                # endsC slot g*15+1+k <- sout[g*924 + 66*k + 65]
                nc.scalar.copy(
                    out=endsC[:].rearrange("p (g c) -> p g c", c=ESTRIDE)[
                        :, :, 1:ESTRIDE
                    ],
                    in_=sout[:].rearrange("p (g c) -> p g c", c=NCOL)[
                        :, :, NCOL - 1 : NCOL
                    ].rearrange("p (g k) c -> p g (k c)", k=14),
                )
                # merged staged scans run continuously across 3 groups, so
                # the interior groups' leading slots hold the predecessor's
                # last end instead of zero
                for m in range(N_STAGED // 3):
                    nc.scalar.copy(
                        out=endsC[:].rearrange("p (g c) -> p g c", c=ESTRIDE)[
                            :, 3 * m + 1 : 3 * m + 3, 0:1
                        ],
                        in_=sout[:, m * 2772 : (m + 1) * 2772].rearrange(
                            "p (g c) -> p g c", c=14 * NCOL
                        )[:, 0:2, 14 * NCOL - 1 : 14 * NCOL],
                    )
                # z2 = diffs of staged ends (on GpSimd; groups are 14 wide
                # so the output lands contiguously at slot g*14+k)
                pf = pfp.tile([128, NG * (ESTRIDE - 1)], F32)
                e3 = endsC[:].rearrange("p (g c) -> p g c", c=ESTRIDE)
                nc.gpsimd.tensor_sub(
                    out=pf[:].rearrange("p (g c) -> p g c", c=ESTRIDE - 1),
                    in0=e3[:, :, 1:ESTRIDE],
                    in1=e3[:, :, 0 : ESTRIDE - 1],
                )
                # final leaky max(0.2*z2, z2) in place, then store
                nc.vector.scalar_tensor_tensor(
                    out=pf[:, 0:P],
                    in0=pf[:, 0:P],
                    scalar=0.2,
                    in1=pf[:, 0:P],
                    op0=mybir.AluOpType.mult,
                    op1=mybir.AluOpType.max,
                )
                nc.gpsimd.dma_start(
                    out=p_out[bt * 128 : (bt + 1) * 128, :], in_=pf[:, 0:P]
                )

            pending = None
            for bt in range(NBT):
                xh = xhp.tile([128, P * 128], FP8, tag="xh")
                load_x(nc.sync, xh, bt)
                front = emit_front(bt, xh)
                if pending is not None:
                    emit_tail(*pending)
                pending = (bt, *front)
            emit_tail(*pending)
    nc.finalize()
    return nc


def _prep_weights(W1, W2):
    W1T = np.ascontiguousarray(np.transpose(W1, (0, 2, 1)))          # [P,V,W]
    u = 0.2 * np.einsum("pvw,pw->pv", W1T, W2).astype(np.float32)    # [P,V]
    wext = np.concatenate(
        [W1_SCALE * W1T, U_SCALE * u[:, :, None], -U_SCALE * u[:, :, None]],
        axis=2,
    )                                                                # [P,V,66]
    wext = np.ascontiguousarray(np.transpose(wext, (1, 0, 2))).reshape(V, P * NCOL)
    wext_f8 = wext.astype(ml_dtypes.float8_e4m3)
    w2e = np.concatenate(
        [
            (0.8 / W1_SCALE) * W2.astype(np.float32),
            np.full((P, 1), 1.0 / U_SCALE, np.float32),
            np.full((P, 1), -1.0 / U_SCALE, np.float32),
        ],
        axis=1,
    ).reshape(1, P * NCOL).astype(ml_dtypes.bfloat16)                # [1, P*66]
    w2ext = np.ascontiguousarray(np.broadcast_to(w2e, (128, P * NCOL)))
    return wext_f8, w2ext


def _prep_x(x):
    """[B, P*V] f32 -> per-core [NBT*128, P*128] fp8 in [bt, v, p, b] order."""
    xq = x.astype(ml_dtypes.float8_e4m3).view(np.uint8)
    xq = xq.reshape(N_CORES, NBT, 128, P, V)         # (core, bt, b, p, v)
    xt = np.ascontiguousarray(xq.transpose(0, 1, 4, 3, 2))  # (core, bt, v, p, b)
    return xt.reshape(N_CORES, NBT * 128, P * 128).view(ml_dtypes.float8_e4m3)


def kernel(x, W1, W2, gamma, beta, Wd, bd):
    global LAST_RESULTS
    x = np.ascontiguousarray(np.asarray(x, dtype=np.float32))
    W1 = np.asarray(W1, dtype=np.float32)
    W2 = np.asarray(W2, dtype=np.float32)

    if "nc" not in _CACHE:
        _CACHE["nc"] = _build_program()
    nc = _CACHE["nc"]

    wext_f8, w2ext = _prep_weights(W1, W2)
    xt = _prep_x(x)
    in_maps = [
        {
            "xt": xt[c],
            "wext": wext_f8,
            "w2ext": w2ext,
        }
        for c in range(N_CORES)
    ]
    res = run_bass_kernel_spmd(nc, in_maps, list(range(N_CORES)))
    LAST_RESULTS = res

    pvals = np.concatenate(
        [res.results[c]["ps"] for c in range(N_CORES)], axis=0
    ).astype(np.float64)                                              # [B, P]

    mean = pvals.mean(axis=0)
    var = pvals.var(axis=0)
    pn = (pvals - mean) / np.sqrt(var + BN_EPS) * np.asarray(gamma, np.float64) \
        + np.asarray(beta, np.float64)
    pn = pn / np.linalg.norm(pn)
    out = 1.0 / (1.0 + np.exp(-(pn @ np.asarray(Wd, np.float64)
                                + np.asarray(bd, np.float64))))
    return out.astype(np.float32)


# revision 51
# speedup vs baseline: 1.0055x; 1.0055x over previous
"""DeepHisCoM Trainium2 kernel (nn_DeepHisCoM_7017976562218).

Math (reference):
    xr = x.reshape(B, P, V)
    z1 = einsum('bpv,pwv->bpw', xr, W1);  h = leaky(z1)          # per-pathway Linear V->W
    z2 = einsum('bpw,pw->bp', h, W2);     pval = leaky(z2)       # per-pathway Linear W->1
    BN(batch stats) -> global L2 normalize -> sigmoid(pn @ Wd + bd)

Device strategy (8 NeuronCores, batch-sharded 2048 rows/core):
    - x is pre-transposed and fp8(e4m3)-quantized on the HOST into
      [bt, v, pathway, batch] layout, so TensorE consumes it directly as
      matmul lhsT (no on-chip transposes, no transpose PSUM->SBUF copies,
      and half the HBM bytes vs bf16).
    - One 66-column matmul per pathway: rhs = [4*W1p^T | +16u | -16u] (fp8)
      with u = 0.2 * W1p^T @ W2p.  leaky(z1) = 0.2*z1 + 0.8*relu(z1), so
      z2 = sum_w relu(z1)*0.8*W2 + (relu(q) - relu(-q)) with q = 0.2*sum_w
      z1*W2 carried exactly by the +/-u columns (relu is positively
      homogeneous, so the 4x/16x fp8-range scales cancel against w2e).
    - Matmuls write f32 to PSUM, 7 pathways per bank, 14 per 2-bank tile.
    - VectorE: fused prefix-scan DVE op (running sum of w2e * relu(h),
      continuous across the group); per-pathway sums are recovered as
      differences of the segment-end columns.  The op carries a
      hand-authored 2X_1PORT microcode variant (2 bf16 pairs/cycle); for
      6 of 10 groups per tile ScalarE stages PSUM f32 -> SBUF bf16 so the
      scan runs at 2x, and those groups are scanned merged 3-at-a-time.
      The remaining groups scan straight from PSUM f32 at 1x, balancing
      Vector vs Scalar load.
    - Ends extraction (ScalarE strided copy), boundary diffs (GpSimd),
      and the final leaky (VectorE) are software-pipelined one batch tile
      behind the feeder ops so no engine queue head-of-line blocks the
      next tile.
    - BN stats + L2 norm + final linear + sigmoid on host (8 MiB, trivial).

fp8 is safe here: BN renormalizes each pathway and the global L2 norm +
sigmoid-around-0.5 crush relative noise; measured rel err stays ~8e-6.
"""

import os
import sys

import numpy as np

for _p in ("/opt/trn_rl_repo",):
    if _p not in sys.path and os.path.isdir(_p):
        sys.path.insert(0, _p)

import ml_dtypes

import concourse.bacc as bacc
import concourse.bass as bass
import concourse.mybir as mybir
from concourse import dve_ops
from concourse.bass_utils import run_bass_kernel_spmd
from concourse.dve_spec import AluOp, Spec, Src0, Src1, Zero, lower, relu, scan
from concourse.dve_uop import (
    ENABLE,
    AluInp,
    DelayInp,
    DveOpSpec,
    InpSel,
    OutPath,
    OutSel,
    Trigger,
    UopConfig,
)
from concourse.dve_uop import AluOp as HwAluOp
from concourse.tile import TileContext


def _build_2x_program():
    """2X_1PORT uop program for the prefix scan: each cycle consumes a
    packed bf16 pair from each source port (w_e,w_o / h_e,h_o), updates the
    accumulator by w_e*relu(h_e) + w_o*relu(h_o), and writes the post-pair
    prefix to BOTH 16-bit output halves.  Even output positions therefore
    hold the pair prefix instead of the element prefix — harmless, since
    only the odd segment-end columns are ever read.

    Mirrors the lower()-generated 1x program's FSM: state0 = one COUNT
    cycle that zeroes the accumulator flop, state1 = steady until
    SRC_TENSOR_DONE.
    """
    lanes = (
        (1, InpSel.SRC_0),      # chain 0: w_e
        (2, InpSel.SRC_1),      # chain 1: h_e
        (3, InpSel.ZERO),       # chain 2: 0.0 (relu operand / acc init)
        (4, InpSel.SRC_0_HI),   # chain 3: w_o
        (5, InpSel.SRC_1_HI),   # chain 4: h_o
    )

    # state 0: one counted cycle, no source consumption, zero the acc flop
    u0 = UopConfig()
    for lane, src in lanes:
        u0.enable_input(src, lane)
    for i in range(5):
        u0.datapath_config[i].pass_through_alu().pass_through_delay(2)
    u0.datapath_config[5].enable_alu(HwAluOp.BYPASS, AluInp.PREV_DELAY_2)
    u0.datapath_config[6].pass_through_alu()
    u0.datapath_config[7].pass_through_alu()
    u0.repeat_count = 1
    u0.trigger = (Trigger.COUNT, Trigger.NONE, Trigger.NONE)
    u0.next_uop = (1, 0, 0)

    # state 1: steady pair loop
    u1 = UopConfig()
    for lane, src in lanes:
        u1.enable_input(src, lane)
    u1.require_inp0 = ENABLE
    u1.require_inp1 = ENABLE
    u1.trigger = (Trigger.SRC_TENSOR_DONE, Trigger.NONE, Trigger.NONE)
    u1.next_uop = (0, 0, 0)
    u1.enable_output(OutSel.ALU_OUT, OutPath.WR0_LO)
    u1.enable_output(OutSel.ALU_OUT, OutPath.WR0_HI)
    b = u1.datapath_config
    # b0: relu_e = max(h_e, 0)
    b[0].enable_alu(HwAluOp.MAX, AluInp.PREV_DELAY_1, AluInp.PREV_DELAY_2)
    b[0].pass_through_delay(0, 2, 3, 4)
    # b1: m_e = w_e * relu_e
    b[1].enable_alu(HwAluOp.MULTIPLY, AluInp.PREV_DELAY_0, AluInp.PREV_ALU_OUT)
    b[1].pass_through_delay(2, 3, 4)
    # b2: relu_o = max(h_o, 0); chain0 <- m_e
    b[2].enable_alu(HwAluOp.MAX, AluInp.PREV_DELAY_4, AluInp.PREV_DELAY_2)
    b[2].enable_delay_from_src(DelayInp.PREV_ALU_OUT, 0)
    b[2].pass_through_delay(3)
    # b3: m_o = w_o * relu_o
    b[3].enable_alu(HwAluOp.MULTIPLY, AluInp.PREV_DELAY_3, AluInp.PREV_ALU_OUT)
    b[3].pass_through_delay(0)
    # b4: pair = m_o + m_e
    b[4].enable_alu(HwAluOp.ADD, AluInp.PREV_ALU_OUT, AluInp.PREV_DELAY_0)
    # b5: acc += pair  (CURR_ALU_OUT feedback, zeroed by state 0)
    b[5].enable_alu(HwAluOp.ADD, AluInp.CURR_ALU_OUT, AluInp.PREV_ALU_OUT)
    # b6/b7: forward acc to the write stage
    b[6].pass_through_alu()
    b[7].pass_through_alu()
    return [u0, u1]


class _DveOp2x(dve_ops.DveOp):
    """DveOp whose compiled spec carries a hand-authored 2X_1PORT program."""

    def compile(self, ver):
        key = (self.name, ver)
        if (r := dve_ops._COMPILE_CACHE.get(key)) is not None:
            return r
        result = DveOpSpec(
            name=self.name,
            opcode=dve_ops.get_dve_sub_opcode(self.name),
            uops=lower(self.spec, ver=ver),
            rd1_en=True,
            uops_2x=_build_2x_program() if ver == "v3" else None,
            perf_max=1,
        )
        got = result.sha(ver)
        if self.uops_sha.get(ver) != got:
            raise ValueError(
                f"{self.name}: lower() output drifted ({ver}: {got} != pinned "
                f"{self.uops_sha.get(ver)!r})."
            )
        dve_ops._COMPILE_CACHE[key] = result
        return result


def _register_prefix_sum_op():
    """Fused DVE op: out[t] = running sum of in0[t] * relu(in1[t]).

    Per-pathway sums are recovered afterwards as differences of the
    segment-boundary columns of the prefix sum.  Ships a hand-authored
    2X_1PORT uop variant (2 bf16 pairs/cycle) alongside the stock 1x one.
    """
    name = "STT_PREFIX_SUM_2X_ANT"
    for op in dve_ops.OPS:
        if op.name == name:
            return op

    def ref(in0, in1, s0, s1, imm2):
        return np.cumsum(in0.astype(np.float32) * np.maximum(in1, 0), axis=-1)

    spec = Spec(body=scan(AluOp.ADD, Src0 * relu(Src1), init=Zero), reference=ref)
    row = dve_ops._CUSTOM_DVE_ROW_BASE + len(dve_ops.OPS)
    sha = DveOpSpec(
        name=name,
        opcode=row,
        uops=lower(spec, ver="v3"),
        rd1_en=True,
        uops_2x=_build_2x_program(),
        perf_max=1,
    ).sha("v3")
    op = _DveOp2x(name, spec, subdim=False, uops_sha={"v3": sha})
    dve_ops.OPS.append(op)
    dve_ops._SUB_OPCODE_FOR_NAME[name] = row
    dve_ops.CUSTOM_DVE_SPECS[name] = op.spec
    return op


PREFIX_SUM_OP = _register_prefix_sum_op()

P, V, W = 128, 128, 64
B = 16384
N_CORES = 8
BSH = B // N_CORES          # 2048 batch rows per core
NBT = BSH // 128            # 16 batch tiles per core
BN_EPS = 1e-5
NCOL = W + 2                # 66: W1^T columns + (+u, -u)
F32 = mybir.dt.float32
BF16 = mybir.dt.bfloat16
FP8 = mybir.dt.float8e4
W1_SCALE = 4.0              # lift W1 out of fp8-subnormal range
U_SCALE = 16.0              # lift u columns out of fp8-subnormal range

# pathway groups per batch tile: 14 pathways per 2-bank PSUM tile (7 per
# 2 KB bank), 9 groups + a 2-pathway tail
GROUPS = [(g * 14, 14) for g in range(9)] + [(126, 2)]
NG = len(GROUPS)
ESTRIDE = 15                # ends-staging slots per group (leading zero + 14)
N_STAGED = 6                # groups per bt staged to SBUF bf16 (2x scan);
                            # the rest scan straight from PSUM f32 at 1x
NXCHUNK = 2                 # x-tile DMA split (compute starts on chunk 1)

_CACHE = {}
LAST_RESULTS = None


def _build_program():
    nc = bacc.Bacc()
    # row = bt*128 + v, col = pathway*128 + batch
    xt_in = nc.declare_dram_parameter("xt", [NBT * 128, P * 128], FP8, isOutput=False)
    wext_in = nc.declare_dram_parameter("wext", [V, P * NCOL], FP8, isOutput=False)
    w2e_in = nc.declare_dram_parameter("w2ext", [128, P * NCOL], BF16, isOutput=False)
    p_out = nc.declare_dram_parameter("ps", [BSH, P], F32, isOutput=True)

    with TileContext(nc) as tc:
        with (
            tc.tile_pool(name="singles", bufs=1) as singles,
            tc.tile_pool(name="xh", bufs=4) as xhp,
            tc.tile_pool(name="hsb", bufs=6) as hsbp,
            tc.tile_pool(name="sout", bufs=4) as soutp,
            tc.tile_pool(name="ends", bufs=3) as endsp,
            tc.tile_pool(name="pf", bufs=3) as pfp,
            tc.tile_pool(name="hps", bufs=4, space="PSUM") as hpsp,
        ):
            wext = singles.tile([V, P * NCOL], FP8)
            nc.sync.dma_start(out=wext[:], in_=wext_in[:, :])
            w2e = singles.tile([128, P * NCOL], BF16)
            # two chunks: the first merged scan only needs the low half
            nc.scalar.dma_start(
                out=w2e[:, : 64 * NCOL], in_=w2e_in[:, : 64 * NCOL]
            )
            nc.scalar.dma_start(
                out=w2e[:, 64 * NCOL :], in_=w2e_in[:, 64 * NCOL :]
            )

            CHW = P * 128 // NXCHUNK

            def load_x(eng, tile, bt):
                # chunked so the first matmul group can start after 1/NXCHUNK
                # of the tile has landed
                for ch in range(NXCHUNK):
                    eng.dma_start(
                        out=tile[:, ch * CHW : (ch + 1) * CHW],
                        in_=xt_in[bt * 128 : (bt + 1) * 128,
                                  ch * CHW : (ch + 1) * CHW],
                    )

            def emit_front(bt, xh):
                """matmuls + staging copies + scans for one batch tile."""
                # ends staging: slot g*15 stays 0 (leading zero per group)
                endsC = endsp.tile([128, NG * ESTRIDE], F32)
                nc.gpsimd.memset(endsC[:], 0.0)
                # per-bt scan output: group gi at column offset gi*924 (the
                # tail only fills 132 of its 924 slots; the rest is junk)
                sout = soutp.tile([128, NG * 14 * NCOL], BF16)
                hsb = None
                for gi, (gs, G) in enumerate(GROUPS):
                    g2 = (G + 1) // 2
                    h_ps = hpsp.tile([128, 1024], F32)
                    for j in range(G):
                        pa = gs + j
                        off = (j // g2) * 512 + (j % g2) * NCOL
                        nc.tensor.matmul(
                            h_ps[:, off : off + NCOL],
                            lhsT=xh[:, pa * 128 : (pa + 1) * 128],
                            rhs=wext[:, pa * NCOL : (pa + 1) * NCOL],
                            start=True,
                            stop=True,
                        )
                    if gi < N_STAGED:
                        # ScalarE: whole-tile PSUM f32 -> SBUF bf16 copy
                        # (contiguous, includes bank pads); the scan then
                        # runs in 2X_1PORT mode on packed bf16 pairs.
                        # Staged groups are scanned merged, 3 at a time.
                        if gi % 3 == 0:
                            hsb = hsbp.tile([128, 3 * 1024], BF16)
                        # only cols [0:974] are ever read by the scan (the
                        # last 50 are bank-1 tail padding) — skip them
                        nc.scalar.copy(
                            out=hsb[:, (gi % 3) * 1024 : (gi % 3) * 1024 + 974],
                            in_=h_ps[:, :974],
                        )
                        if gi % 3 == 2:
                            g0 = gi - 2
                            dve_inst = nc.vector._custom_dve(
                                PREFIX_SUM_OP,
                                out=sout[
                                    :, g0 * 14 * NCOL : (g0 + 3) * 14 * NCOL
                                ].rearrange("p (b c) -> p b c", b=6),
                                in0=w2e[
                                    :, g0 * 14 * NCOL : (g0 + 3) * 14 * NCOL
                                ].rearrange("p (b c) -> p b c", b=6),
                                in1=hsb[:].rearrange("p (b c) -> p b c", b=6)[
                                    :, :, : g2 * NCOL
                                ],
                            )
                            dve_inst.ins.perf_max = 1
                    else:
                        # unstaged: scan reads PSUM f32 directly; mode
                        # detection falls back to the 1x program
                        so = sout[:, gi * 14 * NCOL : gi * 14 * NCOL + G * NCOL]
                        dve_inst = nc.vector._custom_dve(
                            PREFIX_SUM_OP,
                            out=so.rearrange("p (b c) -> p b c", b=2),
                            in0=w2e[:, gs * NCOL : (gs + G) * NCOL].rearrange(
                                "p (b c) -> p b c", b=2
                            ),
                            in1=h_ps[:].rearrange("p (b c) -> p b c", b=2)[
                                :, :, : g2 * NCOL
                            ],
                        )
                        dve_inst.ins.perf_max = 1
                return sout, endsC

            def emit_tail(bt, sout, endsC):
                """ends extraction + diffs + leaky + store, emitted one bt
                late so these queue entries never block the next tile's
                feeder ops on the same engines."""
                # one strided ScalarE copy stages ALL segment-end columns:
                # endsC slot g*15+1+k <- sout[g*924 + 66*k + 65]
                nc.scalar.copy(
                    out=endsC[:].rearrange("p (g c) -> p g c", c=ESTRIDE)[
                        :, :, 1:ESTRIDE
                    ],
                    in_=sout[:].rearrange("p (g c) -> p g c", c=NCOL)[
                        :, :, NCOL - 1 : NCOL
                    ].rearrange("p (g k) c -> p g (k c)", k=14),
                )
                # merged staged scans run continuously across 3 groups, so
                # the interior groups' leading slots hold the predecessor's
                # last end instead of zero
                for m in range(N_STAGED // 3):
                    nc.scalar.copy(
                        out=endsC[:].rearrange("p (g c) -> p g c", c=ESTRIDE)[
                            :, 3 * m + 1 : 3 * m + 3, 0:1
                        ],
                        in_=sout[:, m * 2772 : (m + 1) * 2772].rearrange(
                            "p (g c) -> p g c", c=14 * NCOL
                        )[:, 0:2, 14 * NCOL - 1 : 14 * NCOL],
                    )
                # z2 = diffs of staged ends (on GpSimd; groups are 14 wide
                # so the output lands contiguously at slot g*14+k)
                pf = pfp.tile([128, NG * (ESTRIDE - 1)], F32)
                e3 = endsC[:].rearrange("p (g c) -> p g c", c=ESTRIDE)
                nc.gpsimd.tensor_sub(
                    out=pf[:].rearrange("p (g c) -> p g c", c=ESTRIDE - 1),
                    in0=e3[:, :, 1:ESTRIDE],
                    in1=e3[:, :, 0 : ESTRIDE - 1],
                )
                # final leaky max(0.2*z2, z2) in place, then store
                nc.vector.scalar_tensor_tensor(
                    out=pf[:, 0:P],
                    in0=pf[:, 0:P],
                    scalar=0.2,
                    in1=pf[:, 0:P],
                    op0=mybir.AluOpType.mult,
                    op1=mybir.AluOpType.max,
                )
                nc.gpsimd.dma_start(
                    out=p_out[bt * 128 : (bt + 1) * 128, :], in_=pf[:, 0:P]
                )

            pending = None
            for bt in range(NBT):
                xh = xhp.tile([128, P * 128], FP8, tag="xh")
                load_x(nc.sync, xh, bt)
                front = emit_front(bt, xh)
                if pending is not None:
                    emit_tail(*pending)
                pending = (bt, *front)
            emit_tail(*pending)
    nc.finalize()
    return nc


def _prep_weights(W1, W2):
    W1T = np.ascontiguousarray(np.transpose(W1, (0, 2, 1)))          # [P,V,W]
    u = 0.2 * np.einsum("pvw,pw->pv", W1T, W2).astype(np.float32)    # [P,V]
    wext = np.concatenate(
        [W1_SCALE * W1T, U_SCALE * u[:, :, None], -U_SCALE * u[:, :, None]],
        axis=2,
    )                                                                # [P,V,66]
    wext = np.ascontiguousarray(np.transpose(wext, (1, 0, 2))).reshape(V, P * NCOL)
    wext_f8 = wext.astype(ml_dtypes.float8_e4m3)
    w2e = np.concatenate(
        [
            (0.8 / W1_SCALE) * W2.astype(np.float32),
            np.full((P, 1), 1.0 / U_SCALE, np.float32),
            np.full((P, 1), -1.0 / U_SCALE, np.float32),
        ],
        axis=1,
    ).reshape(1, P * NCOL).astype(ml_dtypes.bfloat16)                # [1, P*66]
    w2ext = np.ascontiguousarray(np.broadcast_to(w2e, (128, P * NCOL)))
    return wext_f8, w2ext


def _prep_x(x):
    """[B, P*V] f32 -> per-core [NBT*128, P*128] fp8 in [bt, v, p, b] order."""
    xq = x.astype(ml_dtypes.float8_e4m3).view(np.uint8)
    xq = xq.reshape(N_CORES, NBT, 128, P, V)         # (core, bt, b, p, v)
    xt = np.ascontiguousarray(xq.transpose(0, 1, 4, 3, 2))  # (core, bt, v, p, b)
    return xt.reshape(N_CORES, NBT * 128, P * 128).view(ml_dtypes.float8_e4m3)


def kernel(x, W1, W2, gamma, beta, Wd, bd):
    global LAST_RESULTS
    x = np.ascontiguousarray(np.asarray(x, dtype=np.float32))
    W1 = np.asarray(W1, dtype=np.float32)
    W2 = np.asarray(W2, dtype=np.float32)

    if "nc" not in _CACHE:
        _CACHE["nc"] = _build_program()
    nc = _CACHE["nc"]

    wext_f8, w2ext = _prep_weights(W1, W2)
    xt = _prep_x(x)
    in_maps = [
        {
            "xt": xt[c],
            "wext": wext_f8,
            "w2ext": w2ext,
        }
        for c in range(N_CORES)
    ]
    res = run_bass_kernel_spmd(nc, in_maps, list(range(N_CORES)))
    LAST_RESULTS = res

    pvals = np.concatenate(
        [res.results[c]["ps"] for c in range(N_CORES)], axis=0
    ).astype(np.float64)                                              # [B, P]

    mean = pvals.mean(axis=0)
    var = pvals.var(axis=0)
    pn = (pvals - mean) / np.sqrt(var + BN_EPS) * np.asarray(gamma, np.float64) \
        + np.asarray(beta, np.float64)
    pn = pn / np.linalg.norm(pn)
    out = 1.0 / (1.0 + np.exp(-(pn @ np.asarray(Wd, np.float64)
                                + np.asarray(bd, np.float64))))
    return out.astype(np.float32)


# revision 52
# speedup vs baseline: 1.0296x; 1.0240x over previous
"""DeepHisCoM Trainium2 kernel (nn_DeepHisCoM_7017976562218).

Math (reference):
    xr = x.reshape(B, P, V)
    z1 = einsum('bpv,pwv->bpw', xr, W1);  h = leaky(z1)          # per-pathway Linear V->W
    z2 = einsum('bpw,pw->bp', h, W2);     pval = leaky(z2)       # per-pathway Linear W->1
    BN(batch stats) -> global L2 normalize -> sigmoid(pn @ Wd + bd)

Device strategy (8 NeuronCores, batch-sharded 2048 rows/core):
    - x is pre-transposed and fp8(e4m3)-quantized on the HOST into
      [bt, v, pathway, batch] layout, so TensorE consumes it directly as
      matmul lhsT (no on-chip transposes, no transpose PSUM->SBUF copies,
      and half the HBM bytes vs bf16).
    - One 66-column matmul per pathway: rhs = [4*W1p^T | +16u | -16u] (fp8)
      with u = 0.2 * W1p^T @ W2p.  leaky(z1) = 0.2*z1 + 0.8*relu(z1), so
      z2 = sum_w relu(z1)*0.8*W2 + (relu(q) - relu(-q)) with q = 0.2*sum_w
      z1*W2 carried exactly by the +/-u columns (relu is positively
      homogeneous, so the 4x/16x fp8-range scales cancel against w2e).
    - Matmuls write f32 to PSUM, 7 pathways per bank, 14 per 2-bank tile.
    - VectorE: fused prefix-scan DVE op (running sum of w2e * relu(h),
      continuous across the group); per-pathway sums are recovered as
      differences of the segment-end columns.  The op carries a
      hand-authored 2X_1PORT microcode variant (2 bf16 pairs/cycle); for
      6 of 10 groups per tile ScalarE stages PSUM f32 -> SBUF bf16 so the
      scan runs at 2x, and those groups are scanned merged 3-at-a-time.
      The remaining groups scan straight from PSUM f32 at 1x, balancing
      Vector vs Scalar load.
    - Ends extraction (ScalarE strided copy), boundary diffs (GpSimd),
      and the final leaky (VectorE) are software-pipelined one batch tile
      behind the feeder ops so no engine queue head-of-line blocks the
      next tile.
    - BN stats + L2 norm + final linear + sigmoid on host (8 MiB, trivial).

fp8 is safe here: BN renormalizes each pathway and the global L2 norm +
sigmoid-around-0.5 crush relative noise; measured rel err stays ~8e-6.
"""

import os
import sys

import numpy as np

for _p in ("/opt/trn_rl_repo",):
    if _p not in sys.path and os.path.isdir(_p):
        sys.path.insert(0, _p)

import ml_dtypes

import concourse.bacc as bacc
import concourse.bass as bass
import concourse.mybir as mybir
from concourse import dve_ops
from concourse.bass_utils import run_bass_kernel_spmd
from concourse.dve_spec import AluOp, Spec, Src0, Src1, Zero, lower, relu, scan
from concourse.dve_uop import (
    ENABLE,
    AluInp,
    DelayInp,
    DveOpSpec,
    InpSel,
    OutPath,
    OutSel,
    Trigger,
    UopConfig,
)
from concourse.dve_uop import AluOp as HwAluOp
from concourse.tile import TileContext


def _build_2x_program():
    """2X_1PORT uop program for the prefix scan: each cycle consumes a
    packed bf16 pair from each source port (w_e,w_o / h_e,h_o), updates the
    accumulator by w_e*relu(h_e) + w_o*relu(h_o), and writes the post-pair
    prefix to BOTH 16-bit output halves.  Even output positions therefore
    hold the pair prefix instead of the element prefix — harmless, since
    only the odd segment-end columns are ever read.

    Mirrors the lower()-generated 1x program's FSM: state0 = one COUNT
    cycle that zeroes the accumulator flop, state1 = steady until
    SRC_TENSOR_DONE.
    """
    lanes = (
        (1, InpSel.SRC_0),      # chain 0: w_e
        (2, InpSel.SRC_1),      # chain 1: h_e
        (3, InpSel.ZERO),       # chain 2: 0.0 (relu operand / acc init)
        (4, InpSel.SRC_0_HI),   # chain 3: w_o
        (5, InpSel.SRC_1_HI),   # chain 4: h_o
    )

    # state 0: one counted cycle, no source consumption, zero the acc flop
    u0 = UopConfig()
    for lane, src in lanes:
        u0.enable_input(src, lane)
    for i in range(5):
        u0.datapath_config[i].pass_through_alu().pass_through_delay(2)
    u0.datapath_config[5].enable_alu(HwAluOp.BYPASS, AluInp.PREV_DELAY_2)
    u0.datapath_config[6].pass_through_alu()
    u0.datapath_config[7].pass_through_alu()
    u0.repeat_count = 1
    u0.trigger = (Trigger.COUNT, Trigger.NONE, Trigger.NONE)
    u0.next_uop = (1, 0, 0)

    # state 1: steady pair loop
    u1 = UopConfig()
    for lane, src in lanes:
        u1.enable_input(src, lane)
    u1.require_inp0 = ENABLE
    u1.require_inp1 = ENABLE
    u1.trigger = (Trigger.SRC_TENSOR_DONE, Trigger.NONE, Trigger.NONE)
    u1.next_uop = (0, 0, 0)
    u1.enable_output(OutSel.ALU_OUT, OutPath.WR0_LO)
    u1.enable_output(OutSel.ALU_OUT, OutPath.WR0_HI)
    b = u1.datapath_config
    # b0: relu_e = max(h_e, 0)
    b[0].enable_alu(HwAluOp.MAX, AluInp.PREV_DELAY_1, AluInp.PREV_DELAY_2)
    b[0].pass_through_delay(0, 2, 3, 4)
    # b1: m_e = w_e * relu_e
    b[1].enable_alu(HwAluOp.MULTIPLY, AluInp.PREV_DELAY_0, AluInp.PREV_ALU_OUT)
    b[1].pass_through_delay(2, 3, 4)
    # b2: relu_o = max(h_o, 0); chain0 <- m_e
    b[2].enable_alu(HwAluOp.MAX, AluInp.PREV_DELAY_4, AluInp.PREV_DELAY_2)
    b[2].enable_delay_from_src(DelayInp.PREV_ALU_OUT, 0)
    b[2].pass_through_delay(3)
    # b3: m_o = w_o * relu_o
    b[3].enable_alu(HwAluOp.MULTIPLY, AluInp.PREV_DELAY_3, AluInp.PREV_ALU_OUT)
    b[3].pass_through_delay(0)
    # b4: pair = m_o + m_e
    b[4].enable_alu(HwAluOp.ADD, AluInp.PREV_ALU_OUT, AluInp.PREV_DELAY_0)
    # b5: acc += pair  (CURR_ALU_OUT feedback, zeroed by state 0)
    b[5].enable_alu(HwAluOp.ADD, AluInp.CURR_ALU_OUT, AluInp.PREV_ALU_OUT)
    # b6/b7: forward acc to the write stage
    b[6].pass_through_alu()
    b[7].pass_through_alu()
    return [u0, u1]


class _DveOp2x(dve_ops.DveOp):
    """DveOp whose compiled spec carries a hand-authored 2X_1PORT program."""

    def compile(self, ver):
        key = (self.name, ver)
        if (r := dve_ops._COMPILE_CACHE.get(key)) is not None:
            return r
        result = DveOpSpec(
            name=self.name,
            opcode=dve_ops.get_dve_sub_opcode(self.name),
            uops=lower(self.spec, ver=ver),
            rd1_en=True,
            uops_2x=_build_2x_program() if ver == "v3" else None,
            perf_max=1,
        )
        got = result.sha(ver)
        if self.uops_sha.get(ver) != got:
            raise ValueError(
                f"{self.name}: lower() output drifted ({ver}: {got} != pinned "
                f"{self.uops_sha.get(ver)!r})."
            )
        dve_ops._COMPILE_CACHE[key] = result
        return result


def _register_prefix_sum_op():
    """Fused DVE op: out[t] = running sum of in0[t] * relu(in1[t]).

    Per-pathway sums are recovered afterwards as differences of the
    segment-boundary columns of the prefix sum.  Ships a hand-authored
    2X_1PORT uop variant (2 bf16 pairs/cycle) alongside the stock 1x one.
    """
    name = "STT_PREFIX_SUM_2X_ANT"
    for op in dve_ops.OPS:
        if op.name == name:
            return op

    def ref(in0, in1, s0, s1, imm2):
        return np.cumsum(in0.astype(np.float32) * np.maximum(in1, 0), axis=-1)

    spec = Spec(body=scan(AluOp.ADD, Src0 * relu(Src1), init=Zero), reference=ref)
    row = dve_ops._CUSTOM_DVE_ROW_BASE + len(dve_ops.OPS)
    sha = DveOpSpec(
        name=name,
        opcode=row,
        uops=lower(spec, ver="v3"),
        rd1_en=True,
        uops_2x=_build_2x_program(),
        perf_max=1,
    ).sha("v3")
    op = _DveOp2x(name, spec, subdim=False, uops_sha={"v3": sha})
    dve_ops.OPS.append(op)
    dve_ops._SUB_OPCODE_FOR_NAME[name] = row
    dve_ops.CUSTOM_DVE_SPECS[name] = op.spec
    return op


PREFIX_SUM_OP = _register_prefix_sum_op()

P, V, W = 128, 128, 64
B = 16384
N_CORES = 8
BSH = B // N_CORES          # 2048 batch rows per core
NBT = BSH // 128            # 16 batch tiles per core
BN_EPS = 1e-5
NCOL = W + 2                # 66: W1^T columns + (+u, -u)
F32 = mybir.dt.float32
BF16 = mybir.dt.bfloat16
FP8 = mybir.dt.float8e4
W1_SCALE = 4.0              # lift W1 out of fp8-subnormal range
U_SCALE = 16.0              # lift u columns out of fp8-subnormal range

# pathway groups per batch tile: 14 pathways per 2-bank PSUM tile (7 per
# 2 KB bank), 9 groups + a 2-pathway tail
GROUPS = [(g * 14, 14) for g in range(9)] + [(126, 2)]
NG = len(GROUPS)
ESTRIDE = 15                # ends-staging slots per group (leading zero + 14)
N_STAGED = 6                # groups per bt staged to SBUF bf16 (2x scan);
                            # the rest scan straight from PSUM f32 at 1x
NXCHUNK = 4                 # x-tile DMA split (compute starts on chunk 1)

_CACHE = {}
LAST_RESULTS = None


def _build_program():
    nc = bacc.Bacc()
    # row = bt*128 + v, col = pathway*128 + batch
    xt_in = nc.declare_dram_parameter("xt", [NBT * 128, P * 128], FP8, isOutput=False)
    wext_in = nc.declare_dram_parameter("wext", [V, P * NCOL], FP8, isOutput=False)
    w2e_in = nc.declare_dram_parameter("w2ext", [128, P * NCOL], BF16, isOutput=False)
    p_out = nc.declare_dram_parameter("ps", [BSH, P], F32, isOutput=True)

    with TileContext(nc) as tc:
        with (
            tc.tile_pool(name="singles", bufs=1) as singles,
            tc.tile_pool(name="xh", bufs=4) as xhp,
            tc.tile_pool(name="hsb", bufs=6) as hsbp,
            tc.tile_pool(name="sout", bufs=4) as soutp,
            tc.tile_pool(name="ends", bufs=3) as endsp,
            tc.tile_pool(name="pf", bufs=3) as pfp,
            tc.tile_pool(name="hps", bufs=4, space="PSUM") as hpsp,
        ):
            wext = singles.tile([V, P * NCOL], FP8)
            nc.sync.dma_start(out=wext[:], in_=wext_in[:, :])
            w2e = singles.tile([128, P * NCOL], BF16)
            # two chunks: the first merged scan only needs the low half
            nc.scalar.dma_start(
                out=w2e[:, : 64 * NCOL], in_=w2e_in[:, : 64 * NCOL]
            )
            nc.scalar.dma_start(
                out=w2e[:, 64 * NCOL :], in_=w2e_in[:, 64 * NCOL :]
            )

            CHW = P * 128 // NXCHUNK

            def load_x(eng, tile, bt):
                # chunked so the first matmul group can start after 1/NXCHUNK
                # of the tile has landed
                for ch in range(NXCHUNK):
                    eng.dma_start(
                        out=tile[:, ch * CHW : (ch + 1) * CHW],
                        in_=xt_in[bt * 128 : (bt + 1) * 128,
                                  ch * CHW : (ch + 1) * CHW],
                    )

            def emit_front(bt, xh):
                """matmuls + staging copies + scans for one batch tile."""
                # ends staging: slot g*15 stays 0 (leading zero per group)
                endsC = endsp.tile([128, NG * ESTRIDE], F32)
                nc.gpsimd.memset(endsC[:], 0.0)
                # per-bt scan output: group gi at column offset gi*924 (the
                # tail only fills 132 of its 924 slots; the rest is junk)
                sout = soutp.tile([128, NG * 14 * NCOL], BF16)
                hsb = None
                for gi, (gs, G) in enumerate(GROUPS):
                    g2 = (G + 1) // 2
                    h_ps = hpsp.tile([128, 1024], F32)
                    for j in range(G):
                        pa = gs + j
                        off = (j // g2) * 512 + (j % g2) * NCOL
                        nc.tensor.matmul(
                            h_ps[:, off : off + NCOL],
                            lhsT=xh[:, pa * 128 : (pa + 1) * 128],
                            rhs=wext[:, pa * NCOL : (pa + 1) * NCOL],
                            start=True,
                            stop=True,
                        )
                    if gi < N_STAGED:
                        # ScalarE: whole-tile PSUM f32 -> SBUF bf16 copy
                        # (contiguous, includes bank pads); the scan then
                        # runs in 2X_1PORT mode on packed bf16 pairs.
                        # Staged groups are scanned merged, 3 at a time.
                        if gi % 3 == 0:
                            hsb = hsbp.tile([128, 3 * 1024], BF16)
                        # only cols [0:974] are ever read by the scan (the
                        # last 50 are bank-1 tail padding) — skip them
                        nc.scalar.copy(
                            out=hsb[:, (gi % 3) * 1024 : (gi % 3) * 1024 + 974],
                            in_=h_ps[:, :974],
                        )
                        if gi % 3 == 2:
                            g0 = gi - 2
                            dve_inst = nc.vector._custom_dve(
                                PREFIX_SUM_OP,
                                out=sout[
                                    :, g0 * 14 * NCOL : (g0 + 3) * 14 * NCOL
                                ].rearrange("p (b c) -> p b c", b=6),
                                in0=w2e[
                                    :, g0 * 14 * NCOL : (g0 + 3) * 14 * NCOL
                                ].rearrange("p (b c) -> p b c", b=6),
                                in1=hsb[:].rearrange("p (b c) -> p b c", b=6)[
                                    :, :, : g2 * NCOL
                                ],
                            )
                            dve_inst.ins.perf_max = 1
                    else:
                        # unstaged: scan reads PSUM f32 directly; mode
                        # detection falls back to the 1x program
                        so = sout[:, gi * 14 * NCOL : gi * 14 * NCOL + G * NCOL]
                        dve_inst = nc.vector._custom_dve(
                            PREFIX_SUM_OP,
                            out=so.rearrange("p (b c) -> p b c", b=2),
                            in0=w2e[:, gs * NCOL : (gs + G) * NCOL].rearrange(
                                "p (b c) -> p b c", b=2
                            ),
                            in1=h_ps[:].rearrange("p (b c) -> p b c", b=2)[
                                :, :, : g2 * NCOL
                            ],
                        )
                        dve_inst.ins.perf_max = 1
                return sout, endsC

            def emit_tail(bt, sout, endsC):
                """ends extraction + diffs + leaky + store, emitted one bt
                late so these queue entries never block the next tile's
                feeder ops on the same engines."""
                # one strided ScalarE copy stages ALL segment-end columns:
                # endsC slot g*15+1+k <- sout[g*924 + 66*k + 65]
                nc.scalar.copy(
                    out=endsC[:].rearrange("p (g c) -> p g c", c=ESTRIDE)[
                        :, :, 1:ESTRIDE
                    ],
                    in_=sout[:].rearrange("p (g c) -> p g c", c=NCOL)[
                        :, :, NCOL - 1 : NCOL
                    ].rearrange("p (g k) c -> p g (k c)", k=14),
                )
                # merged staged scans run continuously across 3 groups, so
                # the interior groups' leading slots hold the predecessor's
                # last end instead of zero
                for m in range(N_STAGED // 3):
                    nc.scalar.copy(
                        out=endsC[:].rearrange("p (g c) -> p g c", c=ESTRIDE)[
                            :, 3 * m + 1 : 3 * m + 3, 0:1
                        ],
                        in_=sout[:, m * 2772 : (m + 1) * 2772].rearrange(
                            "p (g c) -> p g c", c=14 * NCOL
                        )[:, 0:2, 14 * NCOL - 1 : 14 * NCOL],
                    )
                # z2 = diffs of staged ends (on GpSimd; groups are 14 wide
                # so the output lands contiguously at slot g*14+k)
                pf = pfp.tile([128, NG * (ESTRIDE - 1)], F32)
                e3 = endsC[:].rearrange("p (g c) -> p g c", c=ESTRIDE)
                nc.gpsimd.tensor_sub(
                    out=pf[:].rearrange("p (g c) -> p g c", c=ESTRIDE - 1),
                    in0=e3[:, :, 1:ESTRIDE],
                    in1=e3[:, :, 0 : ESTRIDE - 1],
                )
                # final leaky max(0.2*z2, z2) in place, then store
                nc.vector.scalar_tensor_tensor(
                    out=pf[:, 0:P],
                    in0=pf[:, 0:P],
                    scalar=0.2,
                    in1=pf[:, 0:P],
                    op0=mybir.AluOpType.mult,
                    op1=mybir.AluOpType.max,
                )
                nc.gpsimd.dma_start(
                    out=p_out[bt * 128 : (bt + 1) * 128, :], in_=pf[:, 0:P]
                )

            pending = None
            for bt in range(NBT):
                xh = xhp.tile([128, P * 128], FP8, tag="xh")
                load_x(nc.sync, xh, bt)
                front = emit_front(bt, xh)
                if pending is not None:
                    emit_tail(*pending)
                pending = (bt, *front)
            emit_tail(*pending)
    nc.finalize()
    return nc


def _prep_weights(W1, W2):
    W1T = np.ascontiguousarray(np.transpose(W1, (0, 2, 1)))          # [P,V,W]
    u = 0.2 * np.einsum("pvw,pw->pv", W1T, W2).astype(np.float32)    # [P,V]
    wext = np.concatenate(
        [W1_SCALE * W1T, U_SCALE * u[:, :, None], -U_SCALE * u[:, :, None]],
        axis=2,
    )                                                                # [P,V,66]
    wext = np.ascontiguousarray(np.transpose(wext, (1, 0, 2))).reshape(V, P * NCOL)
    wext_f8 = wext.astype(ml_dtypes.float8_e4m3)
    w2e = np.concatenate(
        [
            (0.8 / W1_SCALE) * W2.astype(np.float32),
            np.full((P, 1), 1.0 / U_SCALE, np.float32),
            np.full((P, 1), -1.0 / U_SCALE, np.float32),
        ],
        axis=1,
    ).reshape(1, P * NCOL).astype(ml_dtypes.bfloat16)                # [1, P*66]
    w2ext = np.ascontiguousarray(np.broadcast_to(w2e, (128, P * NCOL)))
    return wext_f8, w2ext


def _prep_x(x):
    """[B, P*V] f32 -> per-core [NBT*128, P*128] fp8 in [bt, v, p, b] order."""
    xq = x.astype(ml_dtypes.float8_e4m3).view(np.uint8)
    xq = xq.reshape(N_CORES, NBT, 128, P, V)         # (core, bt, b, p, v)
    xt = np.ascontiguousarray(xq.transpose(0, 1, 4, 3, 2))  # (core, bt, v, p, b)
    return xt.reshape(N_CORES, NBT * 128, P * 128).view(ml_dtypes.float8_e4m3)


def kernel(x, W1, W2, gamma, beta, Wd, bd):
    global LAST_RESULTS
    x = np.ascontiguousarray(np.asarray(x, dtype=np.float32))
    W1 = np.asarray(W1, dtype=np.float32)
    W2 = np.asarray(W2, dtype=np.float32)

    if "nc" not in _CACHE:
        _CACHE["nc"] = _build_program()
    nc = _CACHE["nc"]

    wext_f8, w2ext = _prep_weights(W1, W2)
    xt = _prep_x(x)
    in_maps = [
        {
            "xt": xt[c],
            "wext": wext_f8,
            "w2ext": w2ext,
        }
        for c in range(N_CORES)
    ]
    res = run_bass_kernel_spmd(nc, in_maps, list(range(N_CORES)))
    LAST_RESULTS = res

    pvals = np.concatenate(
        [res.results[c]["ps"] for c in range(N_CORES)], axis=0
    ).astype(np.float64)                                              # [B, P]

    mean = pvals.mean(axis=0)
    var = pvals.var(axis=0)
    pn = (pvals - mean) / np.sqrt(var + BN_EPS) * np.asarray(gamma, np.float64) \
        + np.asarray(beta, np.float64)
    pn = pn / np.linalg.norm(pn)
    out = 1.0 / (1.0 + np.exp(-(pn @ np.asarray(Wd, np.float64)
                                + np.asarray(bd, np.float64))))
    return out.astype(np.float32)
